# revision 13
# baseline (speedup 1.0000x reference)
"""MPGIN encoder distributed across 8 TRN2 NeuronCores.

Strategy (graph-partition per sharding hint):
 - Nodes sharded batch-aligned: core c owns batches [8c, 8c+8) -> contiguous
   node range (batch is sorted). Edges sharded by dst owner, sorted by
   (dst-window, src-half), padded to a common SPMD schedule.
 - segment_sum via one-hot matmul: for each 512-node window, accumulate
   msg.T @ S into PSUM where S[e, n] = (dstoff[e] == n) built on DVE.
 - Activations kept transposed [dim, nodes] so the GIN MLPs run with
   stationary weights; PE transposes recover row-major for gathers/readouts.
 - x[src] gathers via gpsimd dma_gather (int16 idx) from a replicated
   row-major buffer refreshed per layer by AllGather.
 - Graph/motif readouts via one-hot matmul vs batch ids; per-core partials
   summed on host.
"""
import numpy as np

import concourse.bass as bass
import concourse.mybir as mybir
from concourse.bass_utils import run_bass_kernel_spmd
from concourse.tile import TileContext

P = 128
W = 512            # node window for agg/MLP
G = 16             # gather chunk size in tiles
NCORES = 8
D = 128
NGC, NMC = 3, 2
B = 64
FP = mybir.dt.float32
I16 = mybir.dt.int16


# ---------------------------------------------------------------------------
# walrus workaround: split instructions carrying >1 semaphore waits
def _split_multiwait(nc, max_waits=1):
    ctr = [0]
    f = nc.m.functions[0]
    for bb in f.blocks:
        insts = list(bb.instructions)
        out = []
        changed = False
        for inst in insts:
            si = getattr(inst, "sync_info", None)
            waits = list(si.on_wait) if si is not None else []
            if len(waits) > max_waits:
                keep = waits[-max_waits:]
                for w in waits[:-max_waits]:
                    ctr[0] += 1
                    ev = mybir.InstEventSemaphore(
                        name=f"WSPLIT-{ctr[0]}", engine=inst.engine, ins=[], outs=[]
                    )
                    ev.sync_info = mybir.SyncInfo(on_wait=[w], on_update=[])
                    out.append(ev)
                si.on_wait = keep
                changed = True
            out.append(inst)
        if changed:
            bb.instructions = out


def _ceil(a, b):
    return -(-a // b)


def _pack_idx16(flat):
    """Pack flat int indices into dma_gather idx layout [128, len//16] int16."""
    n = len(flat)
    assert n % 128 == 0
    ef = np.asarray(flat, np.int16).reshape(n // 128, 8, 16)
    idx16 = np.zeros((16, n // 16), np.int16)
    for p in range(16):
        idx16[p] = ef[:, :, p].reshape(-1)
    return np.tile(idx16, (8, 1))


def _pack_cols(vals, ncols, fill):
    """Pack per-slot values [ntiles*128] into [128, ntiles] column layout."""
    v = np.full(ncols * 128, fill, np.float32)
    v[: len(vals)] = vals
    return v.reshape(ncols, 128).T.copy()


def _prep(inputs):
    x = np.asarray(inputs["x"], np.float32)
    ea = np.asarray(inputs["edge_attr"], np.float32)
    eidx = np.asarray(inputs["edge_index"], np.int64)
    batch = np.asarray(inputs["batch"], np.int64)
    node2motif = np.asarray(inputs["node2motif"], np.int64)
    num_motifs = np.asarray(inputs["num_motifs"], np.int64)
    meidx = np.asarray(inputs["motif_edge_index"], np.int64)
    motifid = np.asarray(inputs["motifid"], np.int64)
    emb = np.asarray(inputs["emb"], np.float32)
    N = x.shape[0]
    E = eidx.shape[1]
    M_TOT = int(num_motifs.sum())

    # ---- node shards: batches [8c, 8c+8) ----
    bcounts = np.bincount(batch, minlength=B)
    bstart = np.concatenate([[0], np.cumsum(bcounts)])
    nstart = np.array([bstart[8 * c] for c in range(NCORES)] + [N])
    slen = nstart[1:] - nstart[:-1]
    S_pad = int(_ceil(max(slen.max(), 1), W) * W)
    NW = S_pad // W
    NT = S_pad // P
    NG = NCORES * S_pad
    H = NG // 2
    assert H < 32768

    owner_of_node = np.searchsorted(nstart[1:], np.arange(N), side="right")
    gidx_of_node = owner_of_node * S_pad + (np.arange(N) - nstart[owner_of_node])

    # ---- edges ----
    src, dst = eidx[0], eidx[1]
    eown = owner_of_node[dst]
    ldst = dst - nstart[eown]
    ewin = ldst // W
    esrc_g = gidx_of_node[src]

    # counts[c, w]
    counts = np.zeros((NCORES, NW), np.int64)
    np.add.at(counts, (eown, ewin), 1)
    ntile = _ceil(counts, P).max(axis=0)  # [NW] common schedule
    sched = [(w, int(ntile[w])) for w in range(NW) if ntile[w]]
    T_pad = sum(s[1] for s in sched)

    order = np.lexsort((ewin, eown))
    so_src = esrc_g[order]
    so_ldst = ldst[order]
    so_ea = order  # ea row ids
    cs = np.concatenate([[0], np.cumsum(counts.reshape(-1))])
    cstart = cs[:-1].reshape(NCORES, NW)

    per_core = []
    for c in range(NCORES):
        idxflat = np.zeros(T_pad * P, np.int64)
        dstflat = np.full(T_pad * P, -1.0, np.float32)
        earow = np.full(T_pad * P, -1, np.int64)
        tpos = 0
        for (w, nt) in sched:
            cnt = int(counts[c, w])
            s0 = int(cstart[c, w])
            sl = slice(tpos * P, tpos * P + cnt)
            idxflat[sl] = so_src[s0 : s0 + cnt]
            dstflat[sl] = (so_ldst[s0 : s0 + cnt] % W).astype(np.float32)
            earow[sl] = so_ea[s0 : s0 + cnt]
            tpos += nt
        ea_pack = np.zeros((T_pad * P, D), np.float32)
        valid = earow >= 0
        ea_pack[valid] = ea[earow[valid]]
        ea_pack = ea_pack.reshape(T_pad, P, D).transpose(1, 0, 2).reshape(P, T_pad * D)
        eidx32 = idxflat.reshape(T_pad, P).T.astype(np.int32).copy()
        edstp = dstflat.reshape(T_pad, P).T.copy()

        lc = int(slen[c])
        batchv = _pack_cols(batch[nstart[c] : nstart[c] + lc].astype(np.float32), NT, -1.0)
        per_core.append(
            dict(eidx=eidx32, edst=edstp, eap=ea_pack, batchv=batchv,
                 _idxflat=idxflat)
        )

    # replicated padded x0 rows + per-core transposed shard
    x_rep = np.zeros((NG, D), np.float32)
    x_rep[gidx_of_node] = x
    for c in range(NCORES):
        x0T = np.zeros((D, S_pad), np.float32)
        lc = int(slen[c])
        x0T[:, :lc] = x[nstart[c] : nstart[c] + lc].T
        per_core[c]["x0T"] = x0T
        # layer-0 halo: pre-gathered x0[src] stream in edge-slot order
        idxflat = per_core[c].pop("_idxflat")
        xs0 = x_rep[idxflat]
        per_core[c]["xsrc0"] = (
            xs0.reshape(T_pad, P, D).transpose(1, 0, 2).reshape(P, T_pad * D)
        )

    # ---- motifs ----
    partial = np.concatenate([[0], np.cumsum(num_motifs)[:-1]])
    n2m = node2motif + partial[batch]
    mstart = np.array([int(partial[8 * c]) for c in range(NCORES)] + [M_TOT])
    mslen = mstart[1:] - mstart[:-1]
    M_pad = int(_ceil(max(mslen.max(), 1), P) * P)
    MW = M_pad // P
    MG = NCORES * M_pad
    assert MG < 32768
    owner_of_motif = np.searchsorted(mstart[1:], np.arange(M_TOT), side="right")
    gidx_of_motif = owner_of_motif * M_pad + (np.arange(M_TOT) - mstart[owner_of_motif])
    motif_batch = np.searchsorted(np.cumsum(num_motifs), np.arange(M_TOT), side="right")

    msrc, mdst = meidx[0], meidx[1]
    mown = owner_of_motif[mdst]
    mldst = mdst - mstart[mown]
    mwin = mldst // P
    msrc_g = gidx_of_motif[msrc]
    mcounts = np.zeros((NCORES, MW), np.int64)
    np.add.at(mcounts, (mown, mwin), 1)
    mntile = _ceil(mcounts, P).max(axis=0)  # [MW]
    msched = [(w, int(mntile[w])) for w in range(MW) if mntile[w]]
    MT_pad = sum(s[1] for s in msched)

    morder = np.lexsort((mwin, mown))
    mo_src = msrc_g[morder]
    mo_ldst = mldst[morder]
    mcs = np.concatenate([[0], np.cumsum(mcounts.reshape(-1))])
    mcstart = mcs[:-1].reshape(NCORES, MW)

    embrows = emb[motifid]  # [M_TOT, D]
    for c in range(NCORES):
        midxflat = np.zeros(MT_pad * P, np.int64)
        mdstflat = np.full(MT_pad * P, -1.0, np.float32)
        tpos = 0
        for (w, nt) in msched:
            cnt = int(mcounts[c, w])
            s0 = int(mcstart[c, w])
            sl = slice(tpos * P, tpos * P + cnt)
            midxflat[sl] = mo_src[s0 : s0 + cnt]
            mdstflat[sl] = (mo_ldst[s0 : s0 + cnt] % P).astype(np.float32)
            tpos += nt
        lm = int(mslen[c])
        membT = np.zeros((D, M_pad), np.float32)
        membT[:, :lm] = embrows[mstart[c] : mstart[c] + lm].T
        n2ml = _pack_cols(
            (n2m[nstart[c] : nstart[c] + int(slen[c])] - mstart[c]).astype(np.float32),
            NT, -1.0,
        )
        mbv = _pack_cols(
            motif_batch[mstart[c] : mstart[c] + lm].astype(np.float32), MW, -1.0
        )
        per_core[c].update(
            midx=midxflat.reshape(MT_pad, P).T.astype(np.int32).copy(),
            mdst=mdstflat.reshape(MT_pad, P).T.copy(),
            membT=membT,
            n2ml=n2ml,
            mbv=mbv,
        )

    consts = dict(
        iota512=np.tile(np.arange(W, dtype=np.float32), (P, 1)),
        iota512b=np.tile(np.arange(W, 2 * W, dtype=np.float32), (P, 1)),
        iota128=np.tile(np.arange(P, dtype=np.float32), (P, 1)),
        iota64=np.tile(np.arange(B, dtype=np.float32), (P, 1)),
        ident=np.eye(P, dtype=np.float32),
        gcw1=np.concatenate([np.asarray(inputs["gc_W1"][i], np.float32) for i in range(NGC)], 1),
        gcw2=np.concatenate([np.asarray(inputs["gc_W2"][i], np.float32) for i in range(NGC)], 1),
        gcb1=np.stack([np.asarray(inputs["gc_b1"][i], np.float32) for i in range(NGC)], 1),
        gcb2=np.stack([np.asarray(inputs["gc_b2"][i], np.float32) for i in range(NGC)], 1),
        mcw1=np.concatenate([np.asarray(inputs["mc_W1"][i], np.float32) for i in range(NMC)], 1),
        mcw2=np.concatenate([np.asarray(inputs["mc_W2"][i], np.float32) for i in range(NMC)], 1),
        mcb1=np.stack([np.asarray(inputs["mc_b1"][i], np.float32) for i in range(NMC)], 1),
        mcb2=np.stack([np.asarray(inputs["mc_b2"][i], np.float32) for i in range(NMC)], 1),
        linw=np.concatenate(
            [np.asarray(inputs["lin_W"], np.float32)[k * D : (k + 1) * D, :] for k in range(NGC)],
            axis=1,
        ),
        linb=np.asarray(inputs["lin_b"], np.float32).reshape(D, 1),
        x_rep=x_rep,
    )
    geom = dict(
        S_pad=S_pad, NW=NW, NT=NT, NG=NG, H=H, sched=sched, T_pad=T_pad,
        M_pad=M_pad, MW=MW, MG=MG, msched=msched, MT_pad=MT_pad,
    )
    return consts, per_core, geom


def _build(consts, geom):
    S_pad, NW, NT, NG, H = geom["S_pad"], geom["NW"], geom["NT"], geom["NG"], geom["H"]
    sched, T_pad = geom["sched"], geom["T_pad"]
    M_pad, MW, MG = geom["M_pad"], geom["MW"], geom["MG"]
    msched, MT_pad = geom["msched"], geom["MT_pad"]

    nc = bass.Bass(num_devices=NCORES)
    dp = lambda n, s, dt=FP: nc.declare_dram_parameter(n, list(s), dt, isOutput=False)

    ins = {}
    for n, a in consts.items():
        ins[n] = dp(n, a.shape)
    eidx_d = dp("eidx", [P, T_pad], mybir.dt.int32)
    edst_d = dp("edst", [P, T_pad])
    eap_d = dp("eap", [P, T_pad * D])
    batchv_d = dp("batchv", [P, NT])
    xsrc0_d = dp("xsrc0", [P, T_pad * D])
    x0T_d = dp("x0T", [D, S_pad])
    midx_d = dp("midx", [P, MT_pad], mybir.dt.int32)
    mdst_d = dp("mdst", [P, MT_pad])
    membT_d = dp("membT", [D, M_pad])
    n2ml_d = dp("n2ml", [P, NT])
    mbv_d = dp("mbv", [P, MW])

    xg_o = nc.declare_dram_parameter("xg_part", [B, NGC * D], FP, isOutput=True)
    xm_o = nc.declare_dram_parameter("xm_part", [B, NMC * D], FP, isOutput=True)

    ag = [nc.dram_tensor(f"ag{i}", [NG, D], FP, addr_space="Shared") for i in range(2)]
    xr = [nc.dram_tensor(f"xr{i}", [S_pad, D], FP) for i in range(2)]
    xt = [nc.dram_tensor(f"xt{i}", [D, S_pad], FP) for i in range(NGC)]
    mrows = [nc.dram_tensor(f"mrows{i}", [M_pad, D], FP) for i in range(2)]
    agm = [nc.dram_tensor(f"agm{i}", [MG, D], FP, addr_space="Shared") for i in range(2)]

    RG = [list(range(NCORES))]

    regcache = {}

    def nidx_reg(v):
        if v not in regcache:
            regcache[v] = nc.gpsimd.to_reg(v)
        return regcache[v]

    with TileContext(nc) as tc:
        with (
            tc.tile_pool(name="const", bufs=1) as cp,
            tc.tile_pool(name="gat", bufs=3) as gp,
            tc.tile_pool(name="eat", bufs=3) as ep,
            tc.tile_pool(name="sel", bufs=4) as sp,
            tc.tile_pool(name="win", bufs=3) as wp,
            tc.tile_pool(name="rows", bufs=3) as rp,
            tc.tile_pool(name="acc", bufs=1) as ap,
            tc.tile_pool(name="pagg", bufs=2, space="PSUM") as pagg_p,
            tc.tile_pool(name="pmlp", bufs=2, space="PSUM") as pmlp_p,
            tc.tile_pool(name="ptr", bufs=2, space="PSUM") as ptr_p,
            tc.tile_pool(name="pacc", bufs=1, space="PSUM") as pacc_p,
        ):
            # resident constants
            C = {}
            for n, a in consts.items():
                if n == "x_rep":
                    continue
                t = cp.tile(list(a.shape), FP, tag=n)
                nc.sync.dma_start(out=t[:], in_=ins[n][:])
                C[n] = t
            eidx_t = cp.tile([P, T_pad], mybir.dt.int32, tag="eidx")
            nc.sync.dma_start(out=eidx_t[:], in_=eidx_d[:])
            edst_t = cp.tile([P, T_pad], FP, tag="edst")
            nc.sync.dma_start(out=edst_t[:], in_=edst_d[:])
            batchv_t = cp.tile([P, NT], FP, tag="batchv")
            nc.sync.dma_start(out=batchv_t[:], in_=batchv_d[:])
            n2ml_t = cp.tile([P, NT], FP, tag="n2ml")
            nc.sync.dma_start(out=n2ml_t[:], in_=n2ml_d[:])
            midx_t = cp.tile([P, MT_pad], mybir.dt.int32, tag="midx")
            nc.sync.dma_start(out=midx_t[:], in_=midx_d[:])
            mdst_t = cp.tile([P, MT_pad], FP, tag="mdst")
            nc.sync.dma_start(out=mdst_t[:], in_=mdst_d[:])
            mbv_t = cp.tile([P, MW], FP, tag="mbv")
            nc.sync.dma_start(out=mbv_t[:], in_=mbv_d[:])
            membT_t = cp.tile([D, M_pad], FP, tag="membT")
            nc.sync.dma_start(out=membT_t[:], in_=membT_d[:])

            xg_sb = ap.tile([B, NGC * D], FP, tag="xg")
            xm_sb = ap.tile([B, NMC * D], FP, tag="xm")

            # window -> list of (tpos, half, nt) runs; chunked to <= G tiles
            win_chunks = [[] for _ in range(NW)]
            tpos = 0
            for (w, nt) in sched:
                off = 0
                while off < nt:
                    n = min(G, nt - off)
                    win_chunks[w].append((tpos + off, n))
                    off += n
                tpos += nt
            win_tiles = [sum(c[1] for c in win_chunks[w]) for w in range(NW)]

            # ---------------- Phase A: 3 GINE layers ----------------
            for i in range(NGC):
                src_d = ins["x_rep"] if i == 0 else ag[i - 1]
                prevT = x0T_d if i == 0 else xt[i - 1]
                pxg = pacc_p.tile([B, D], FP, tag="acc")
                for w in range(NW):
                    ntw = win_tiles[w]
                    pagg = pagg_p.tile([P, W], FP, tag="agg")
                    ti = 0
                    for (t0, nt) in win_chunks[w]:
                        eat = ep.tile([P, G * D], FP, tag="eat")
                        nc.sync.dma_start(
                            out=eat[:, : nt * D], in_=eap_d[:, t0 * D : (t0 + nt) * D]
                        )
                        if i == 0:
                            gx0 = gp.tile([P, G * D], FP, tag="gx0")
                            nc.sync.dma_start(
                                out=gx0[:, : nt * D],
                                in_=xsrc0_d[:, t0 * D : (t0 + nt) * D],
                            )
                            nc.vector.tensor_add(
                                out=eat[:, : nt * D], in0=eat[:, : nt * D],
                                in1=gx0[:, : nt * D],
                            )
                            nc.scalar.activation(
                                out=eat[:, : nt * D], in_=eat[:, : nt * D],
                                func=mybir.ActivationFunctionType.Relu,
                            )
                        else:
                            for t in range(nt):
                                gx = gp.tile([P, D], FP, tag="gx")
                                nc.gpsimd.indirect_dma_start(
                                    out=gx[:], out_offset=None, in_=src_d[:],
                                    in_offset=bass.IndirectOffsetOnAxis(
                                        ap=eidx_t[:, t0 + t : t0 + t + 1], axis=0
                                    ),
                                )
                                nc.vector.tensor_add(
                                    out=eat[:, t * D : (t + 1) * D],
                                    in0=eat[:, t * D : (t + 1) * D], in1=gx[:],
                                )
                                nc.scalar.activation(
                                    out=eat[:, t * D : (t + 1) * D],
                                    in_=eat[:, t * D : (t + 1) * D],
                                    func=mybir.ActivationFunctionType.Relu,
                                )
                        for t in range(nt):
                            S = sp.tile([P, W], FP, tag="S")
                            nc.vector.tensor_tensor(
                                out=S[:],
                                in0=edst_t[:, t0 + t : t0 + t + 1].to_broadcast([P, W]),
                                in1=C["iota512"][:],
                                op=mybir.AluOpType.is_equal,
                            )
                            nc.tensor.matmul(
                                out=pagg[:], lhsT=eat[:, t * D : (t + 1) * D], rhs=S[:],
                                start=(ti == 0), stop=(ti == ntw - 1),
                            )
                            ti += 1
                    xw = wp.tile([P, W], FP, tag="xw")
                    nc.sync.dma_start(out=xw[:], in_=prevT[:, w * W : (w + 1) * W])
                    tT = wp.tile([P, W], FP, tag="tT")
                    if ntw:
                        nc.vector.tensor_add(out=tT[:], in0=xw[:], in1=pagg[:])
                    else:
                        nc.vector.tensor_copy(out=tT[:], in_=xw[:])
                    p1 = pmlp_p.tile([P, W], FP, tag="pm")
                    nc.tensor.matmul(
                        out=p1[:], lhsT=C["gcw1"][:, i * D : (i + 1) * D], rhs=tT[:],
                        start=True, stop=True,
                    )
                    h1 = wp.tile([P, W], FP, tag="h1")
                    nc.scalar.activation(
                        out=h1[:], in_=p1[:], func=mybir.ActivationFunctionType.Relu,
                        bias=C["gcb1"][:, i : i + 1],
                    )
                    p2 = pmlp_p.tile([P, W], FP, tag="pm")
                    nc.tensor.matmul(
                        out=p2[:], lhsT=C["gcw2"][:, i * D : (i + 1) * D], rhs=h1[:],
                        start=True, stop=True,
                    )
                    xnT = wp.tile([P, W], FP, tag="xnT")
                    nc.scalar.activation(
                        out=xnT[:], in_=p2[:], func=mybir.ActivationFunctionType.Relu,
                        bias=C["gcb2"][:, i : i + 1],
                    )
                    nc.sync.dma_start(out=xt[i][:, w * W : (w + 1) * W], in_=xnT[:])
                    for q in range(4):
                        pt = ptr_p.tile([P, P], FP, tag="pt")
                        nc.tensor.transpose(
                            out=pt[:], in_=xnT[:, q * P : (q + 1) * P], identity=C["ident"][:]
                        )
                        rows = rp.tile([P, P], FP, tag="rows")
                        nc.vector.tensor_copy(out=rows[:], in_=pt[:])
                        if i < 2:
                            nc.sync.dma_start(
                                out=xr[i][w * W + q * P : w * W + (q + 1) * P, :],
                                in_=rows[:],
                            )
                        col = w * 4 + q
                        Sb = sp.tile([P, B], FP, tag="Sb")
                        nc.vector.tensor_tensor(
                            out=Sb[:],
                            in0=batchv_t[:, col : col + 1].to_broadcast([P, B]),
                            in1=C["iota64"][:],
                            op=mybir.AluOpType.is_equal,
                        )
                        nc.tensor.matmul(
                            out=pxg[:], lhsT=Sb[:], rhs=rows[:],
                            start=(col == 0), stop=(col == NT - 1),
                        )
                nc.vector.tensor_copy(out=xg_sb[:, i * D : (i + 1) * D], in_=pxg[:])
                if i < 2:
                    nc.gpsimd.collective_compute(
                        "AllGather", mybir.AluOpType.bypass, replica_groups=RG,
                        ins=[xr[i][:]], outs=[ag[i][:]],
                    )
            nc.sync.dma_start(out=xg_o[:], in_=xg_sb[:])

            # ---------------- Phase C: motif features ----------------
            pxmh = [pacc_p.tile([P, W], FP, tag="acc" if mw == 0 else "acc2", name=f"pxmh{mw}") for mw in range(2)]
            for w in range(NW):
                pC = pmlp_p.tile([P, W], FP, tag="pm")
                for k in range(NGC):
                    xk = wp.tile([P, W], FP, tag="xw")
                    nc.sync.dma_start(out=xk[:], in_=xt[k][:, w * W : (w + 1) * W])
                    nc.tensor.matmul(
                        out=pC[:], lhsT=C["linw"][:, k * D : (k + 1) * D], rhs=xk[:],
                        start=(k == 0), stop=(k == NGC - 1),
                    )
                xmnT = wp.tile([P, W], FP, tag="h1")
                nc.scalar.activation(
                    out=xmnT[:], in_=pC[:], func=mybir.ActivationFunctionType.Relu,
                    bias=C["linb"][:],
                )
                for q in range(4):
                    pt = ptr_p.tile([P, P], FP, tag="pt")
                    nc.tensor.transpose(
                        out=pt[:], in_=xmnT[:, q * P : (q + 1) * P], identity=C["ident"][:]
                    )
                    rows = rp.tile([P, P], FP, tag="rows")
                    nc.vector.tensor_copy(out=rows[:], in_=pt[:])
                    col = w * 4 + q
                    for mw in range(2):
                        Sm = sp.tile([P, W], FP, tag="S")
                        nc.vector.tensor_tensor(
                            out=Sm[:],
                            in0=n2ml_t[:, col : col + 1].to_broadcast([P, W]),
                            in1=C["iota512" if mw == 0 else "iota512b"][:],
                            op=mybir.AluOpType.is_equal,
                        )
                        nc.tensor.matmul(
                            out=pxmh[mw][:], lhsT=rows[:], rhs=Sm[:],
                            start=(col == 0), stop=(col == NT - 1),
                        )
            xmhT = ap.tile([D, M_pad], FP, tag="xmhT")
            nc.vector.tensor_add(
                out=xmhT[:, :W], in0=pxmh[0][:], in1=membT_t[:, :W]
            )
            if M_pad > W:
                nc.vector.tensor_add(
                    out=xmhT[:, W:M_pad], in0=pxmh[1][:, : M_pad - W],
                    in1=membT_t[:, W:M_pad],
                )
            for mw in range(MW):
                pt = ptr_p.tile([P, P], FP, tag="pt")
                nc.tensor.transpose(
                    out=pt[:], in_=xmhT[:, mw * P : (mw + 1) * P], identity=C["ident"][:]
                )
                rows = rp.tile([P, P], FP, tag="rows")
                nc.vector.tensor_copy(out=rows[:], in_=pt[:])
                nc.sync.dma_start(out=mrows[0][mw * P : (mw + 1) * P, :], in_=rows[:])
            nc.gpsimd.collective_compute(
                "AllGather", mybir.AluOpType.bypass, replica_groups=RG,
                ins=[mrows[0][:]], outs=[agm[0][:]],
            )

            # ---------------- motif GIN layers ----------------
            mwin_chunks = [[] for _ in range(MW)]
            tpos = 0
            for (w, nt) in msched:
                off = 0
                while off < nt:
                    n = min(G, nt - off)
                    mwin_chunks[w].append((tpos + off, n))
                    off += n
                tpos += nt
            mwin_tiles = [sum(c[1] for c in mwin_chunks[w]) for w in range(MW)]

            hT_prev = xmhT
            for i in range(NMC):
                pxm = pacc_p.tile([B, D], FP, tag="acc")
                hT_new = ap.tile([D, M_pad], FP, tag=f"mh{i}T")
                for w in range(MW):
                    ntw = mwin_tiles[w]
                    pagg = pagg_p.tile([P, P], FP, tag="agg")
                    ti = 0
                    for (t0, nt) in mwin_chunks[w]:
                        gm = gp.tile([P, G * D], FP, tag="gx")
                        for t in range(nt):
                            nc.gpsimd.indirect_dma_start(
                                out=gm[:, t * D : (t + 1) * D], out_offset=None,
                                in_=agm[i][:],
                                in_offset=bass.IndirectOffsetOnAxis(
                                    ap=midx_t[:, t0 + t : t0 + t + 1], axis=0
                                ),
                            )
                        for t in range(nt):
                            Sm = sp.tile([P, P], FP, tag="Sm128")
                            nc.vector.tensor_tensor(
                                out=Sm[:],
                                in0=mdst_t[:, t0 + t : t0 + t + 1].to_broadcast([P, P]),
                                in1=C["iota128"][:],
                                op=mybir.AluOpType.is_equal,
                            )
                            nc.tensor.matmul(
                                out=pagg[:], lhsT=gm[:, t * D : (t + 1) * D], rhs=Sm[:],
                                start=(ti == 0), stop=(ti == ntw - 1),
                            )
                            ti += 1
                    tT = wp.tile([P, P], FP, tag="mtT")
                    if ntw:
                        nc.vector.tensor_add(
                            out=tT[:], in0=hT_prev[:, w * P : (w + 1) * P], in1=pagg[:]
                        )
                    else:
                        nc.vector.tensor_copy(out=tT[:], in_=hT_prev[:, w * P : (w + 1) * P])
                    p1 = pmlp_p.tile([P, P], FP, tag="pm")
                    nc.tensor.matmul(
                        out=p1[:], lhsT=C["mcw1"][:, i * D : (i + 1) * D], rhs=tT[:],
                        start=True, stop=True,
                    )
                    h1 = wp.tile([P, P], FP, tag="mh1")
                    nc.scalar.activation(
                        out=h1[:], in_=p1[:], func=mybir.ActivationFunctionType.Relu,
                        bias=C["mcb1"][:, i : i + 1],
                    )
                    p2 = pmlp_p.tile([P, P], FP, tag="pm")
                    nc.tensor.matmul(
                        out=p2[:], lhsT=C["mcw2"][:, i * D : (i + 1) * D], rhs=h1[:],
                        start=True, stop=True,
                    )
                    nc.scalar.activation(
                        out=hT_new[:, w * P : (w + 1) * P], in_=p2[:],
                        func=mybir.ActivationFunctionType.Relu,
                        bias=C["mcb2"][:, i : i + 1],
                    )
                    pt = ptr_p.tile([P, P], FP, tag="pt")
                    nc.tensor.transpose(
                        out=pt[:], in_=hT_new[:, w * P : (w + 1) * P],
                        identity=C["ident"][:],
                    )
                    rows = rp.tile([P, P], FP, tag="rows")
                    nc.vector.tensor_copy(out=rows[:], in_=pt[:])
                    if i == 0:
                        nc.sync.dma_start(
                            out=mrows[1][w * P : (w + 1) * P, :], in_=rows[:]
                        )
                    Smb = sp.tile([P, B], FP, tag="Sb")
                    nc.vector.tensor_tensor(
                        out=Smb[:],
                        in0=mbv_t[:, w : w + 1].to_broadcast([P, B]),
                        in1=C["iota64"][:],
                        op=mybir.AluOpType.is_equal,
                    )
                    nc.tensor.matmul(
                        out=pxm[:], lhsT=Smb[:], rhs=rows[:],
                        start=(w == 0), stop=(w == MW - 1),
                    )
                nc.vector.tensor_copy(out=xm_sb[:, i * D : (i + 1) * D], in_=pxm[:])
                if i == 0:
                    nc.gpsimd.collective_compute(
                        "AllGather", mybir.AluOpType.bypass, replica_groups=RG,
                        ins=[mrows[1][:]], outs=[agm[1][:]],
                    )
                hT_prev = hT_new
            nc.sync.dma_start(out=xm_o[:], in_=xm_sb[:])

    _split_multiwait(nc)
    return nc


def kernel(**inputs):
    consts, per_core, geom = _prep(inputs)
    nc = _build(consts, geom)
    in_maps = []
    for c in range(NCORES):
        m = dict(consts)
        m.update(per_core[c])
        in_maps.append({k: np.ascontiguousarray(v) for k, v in m.items()})
    res = run_bass_kernel_spmd(nc, in_maps, list(range(NCORES)))
    xg = np.zeros((B, NGC * D), np.float32)
    xm = np.zeros((B, NMC * D), np.float32)
    for r in res.results:
        xg += r["xg_part"]
        xm += r["xm_part"]
    kernel._last_exec_ns = res.exec_time_ns
    return xm, xg


# revision 14
# speedup vs baseline: 1.0344x; 1.0344x over previous
"""MPGIN encoder distributed across 8 TRN2 NeuronCores.

Strategy (graph-partition per sharding hint):
 - Nodes sharded batch-aligned: core c owns batches [8c, 8c+8) -> contiguous
   node range (batch is sorted). Edges sharded by dst owner, sorted by
   (dst-window, src-half), padded to a common SPMD schedule.
 - segment_sum via one-hot matmul: for each 512-node window, accumulate
   msg.T @ S into PSUM where S[e, n] = (dstoff[e] == n) built on DVE.
 - Activations kept transposed [dim, nodes] so the GIN MLPs run with
   stationary weights; PE transposes recover row-major for gathers/readouts.
 - x[src] gathers via gpsimd dma_gather (int16 idx) from a replicated
   row-major buffer refreshed per layer by AllGather.
 - Graph/motif readouts via one-hot matmul vs batch ids; per-core partials
   summed on host.
"""
import numpy as np

import concourse.bass as bass
import concourse.mybir as mybir
from concourse.bass_utils import run_bass_kernel_spmd
from concourse.tile import TileContext

P = 128
W = 512            # node window for agg/MLP
G = 16             # gather chunk size in tiles
NCORES = 8
D = 128
NGC, NMC = 3, 2
B = 64
FP = mybir.dt.float32
I16 = mybir.dt.int16


# ---------------------------------------------------------------------------
# walrus workaround: split instructions carrying >1 semaphore waits
def _split_multiwait(nc, max_waits=1):
    ctr = [0]
    f = nc.m.functions[0]
    for bb in f.blocks:
        insts = list(bb.instructions)
        out = []
        changed = False
        for inst in insts:
            si = getattr(inst, "sync_info", None)
            waits = list(si.on_wait) if si is not None else []
            if len(waits) > max_waits:
                keep = waits[-max_waits:]
                for w in waits[:-max_waits]:
                    ctr[0] += 1
                    ev = mybir.InstEventSemaphore(
                        name=f"WSPLIT-{ctr[0]}", engine=inst.engine, ins=[], outs=[]
                    )
                    ev.sync_info = mybir.SyncInfo(on_wait=[w], on_update=[])
                    out.append(ev)
                si.on_wait = keep
                changed = True
            out.append(inst)
        if changed:
            bb.instructions = out


def _ceil(a, b):
    return -(-a // b)


def _pack_idx16(flat):
    """Pack flat int indices into dma_gather idx layout [128, len//16] int16."""
    n = len(flat)
    assert n % 128 == 0
    ef = np.asarray(flat, np.int16).reshape(n // 128, 8, 16)
    idx16 = np.zeros((16, n // 16), np.int16)
    for p in range(16):
        idx16[p] = ef[:, :, p].reshape(-1)
    return np.tile(idx16, (8, 1))


def _pack_cols(vals, ncols, fill):
    """Pack per-slot values [ntiles*128] into [128, ntiles] column layout."""
    v = np.full(ncols * 128, fill, np.float32)
    v[: len(vals)] = vals
    return v.reshape(ncols, 128).T.copy()


def _prep(inputs):
    x = np.asarray(inputs["x"], np.float32)
    ea = np.asarray(inputs["edge_attr"], np.float32)
    eidx = np.asarray(inputs["edge_index"], np.int64)
    batch = np.asarray(inputs["batch"], np.int64)
    node2motif = np.asarray(inputs["node2motif"], np.int64)
    num_motifs = np.asarray(inputs["num_motifs"], np.int64)
    meidx = np.asarray(inputs["motif_edge_index"], np.int64)
    motifid = np.asarray(inputs["motifid"], np.int64)
    emb = np.asarray(inputs["emb"], np.float32)
    N = x.shape[0]
    E = eidx.shape[1]
    M_TOT = int(num_motifs.sum())

    # ---- node shards: batches [8c, 8c+8) ----
    bcounts = np.bincount(batch, minlength=B)
    bstart = np.concatenate([[0], np.cumsum(bcounts)])
    nstart = np.array([bstart[8 * c] for c in range(NCORES)] + [N])
    slen = nstart[1:] - nstart[:-1]
    S_pad = int(_ceil(max(slen.max(), 1), W) * W)
    NW = S_pad // W
    NT = S_pad // P
    NG = NCORES * S_pad
    H = NG // 2
    assert H < 32768

    owner_of_node = np.searchsorted(nstart[1:], np.arange(N), side="right")
    gidx_of_node = owner_of_node * S_pad + (np.arange(N) - nstart[owner_of_node])

    # ---- edges ----
    src, dst = eidx[0], eidx[1]
    eown = owner_of_node[dst]
    ldst = dst - nstart[eown]
    ewin = ldst // P
    esrc_g = gidx_of_node[src]

    # counts[c, w] over 128-node windows
    counts = np.zeros((NCORES, NT), np.int64)
    np.add.at(counts, (eown, ewin), 1)
    ntile = _ceil(counts, P).max(axis=0)  # [NT] common schedule
    sched = [(w, int(ntile[w])) for w in range(NT) if ntile[w]]
    T_pad = sum(s[1] for s in sched)

    order = np.lexsort((ewin, eown))
    so_src = esrc_g[order]
    so_ldst = ldst[order]
    so_ea = order  # ea row ids
    cs = np.concatenate([[0], np.cumsum(counts.reshape(-1))])
    cstart = cs[:-1].reshape(NCORES, NT)

    per_core = []
    for c in range(NCORES):
        idxflat = np.zeros(T_pad * P, np.int64)
        dstflat = np.full(T_pad * P, -1.0, np.float32)
        earow = np.full(T_pad * P, -1, np.int64)
        tpos = 0
        for (w, nt) in sched:
            cnt = int(counts[c, w])
            s0 = int(cstart[c, w])
            sl = slice(tpos * P, tpos * P + cnt)
            idxflat[sl] = so_src[s0 : s0 + cnt]
            dstflat[sl] = (so_ldst[s0 : s0 + cnt] % P).astype(np.float32)
            earow[sl] = so_ea[s0 : s0 + cnt]
            tpos += nt
        ea_pack = np.zeros((T_pad * P, D), np.float32)
        valid = earow >= 0
        ea_pack[valid] = ea[earow[valid]]
        ea_pack = ea_pack.reshape(T_pad, P, D).transpose(1, 0, 2).reshape(P, T_pad * D)
        eidx32 = idxflat.reshape(T_pad, P).T.astype(np.int32).copy()
        edstp = dstflat.reshape(T_pad, P).T.copy()

        lc = int(slen[c])
        batchv = _pack_cols(batch[nstart[c] : nstart[c] + lc].astype(np.float32), NT, -1.0)
        per_core.append(
            dict(eidx=eidx32, edst=edstp, eap=ea_pack, batchv=batchv,
                 _idxflat=idxflat)
        )

    # replicated padded x0 rows + per-core transposed shard
    x_rep = np.zeros((NG, D), np.float32)
    x_rep[gidx_of_node] = x
    for c in range(NCORES):
        x0T = np.zeros((D, S_pad), np.float32)
        lc = int(slen[c])
        x0T[:, :lc] = x[nstart[c] : nstart[c] + lc].T
        per_core[c]["x0T"] = x0T
        # layer-0 halo: pre-gathered x0[src] stream in edge-slot order
        idxflat = per_core[c].pop("_idxflat")
        xs0 = x_rep[idxflat]
        per_core[c]["xsrc0"] = (
            xs0.reshape(T_pad, P, D).transpose(1, 0, 2).reshape(P, T_pad * D)
        )

    # ---- motifs ----
    partial = np.concatenate([[0], np.cumsum(num_motifs)[:-1]])
    n2m = node2motif + partial[batch]
    mstart = np.array([int(partial[8 * c]) for c in range(NCORES)] + [M_TOT])
    mslen = mstart[1:] - mstart[:-1]
    M_pad = int(_ceil(max(mslen.max(), 1), P) * P)
    MW = M_pad // P
    MG = NCORES * M_pad
    assert MG < 32768
    owner_of_motif = np.searchsorted(mstart[1:], np.arange(M_TOT), side="right")
    gidx_of_motif = owner_of_motif * M_pad + (np.arange(M_TOT) - mstart[owner_of_motif])
    motif_batch = np.searchsorted(np.cumsum(num_motifs), np.arange(M_TOT), side="right")

    msrc, mdst = meidx[0], meidx[1]
    mown = owner_of_motif[mdst]
    mldst = mdst - mstart[mown]
    mwin = mldst // P
    msrc_g = gidx_of_motif[msrc]
    mcounts = np.zeros((NCORES, MW), np.int64)
    np.add.at(mcounts, (mown, mwin), 1)
    mntile = _ceil(mcounts, P).max(axis=0)  # [MW]
    msched = [(w, int(mntile[w])) for w in range(MW) if mntile[w]]
    MT_pad = sum(s[1] for s in msched)

    morder = np.lexsort((mwin, mown))
    mo_src = msrc_g[morder]
    mo_ldst = mldst[morder]
    mcs = np.concatenate([[0], np.cumsum(mcounts.reshape(-1))])
    mcstart = mcs[:-1].reshape(NCORES, MW)

    embrows = emb[motifid]  # [M_TOT, D]
    for c in range(NCORES):
        midxflat = np.zeros(MT_pad * P, np.int64)
        mdstflat = np.full(MT_pad * P, -1.0, np.float32)
        tpos = 0
        for (w, nt) in msched:
            cnt = int(mcounts[c, w])
            s0 = int(mcstart[c, w])
            sl = slice(tpos * P, tpos * P + cnt)
            midxflat[sl] = mo_src[s0 : s0 + cnt]
            mdstflat[sl] = (mo_ldst[s0 : s0 + cnt] % P).astype(np.float32)
            tpos += nt
        lm = int(mslen[c])
        membT = np.zeros((D, M_pad), np.float32)
        membT[:, :lm] = embrows[mstart[c] : mstart[c] + lm].T
        n2ml = _pack_cols(
            (n2m[nstart[c] : nstart[c] + int(slen[c])] - mstart[c]).astype(np.float32),
            NT, -1.0,
        )
        mbv = _pack_cols(
            motif_batch[mstart[c] : mstart[c] + lm].astype(np.float32), MW, -1.0
        )
        per_core[c].update(
            midx=midxflat.reshape(MT_pad, P).T.astype(np.int32).copy(),
            mdst=mdstflat.reshape(MT_pad, P).T.copy(),
            membT=membT,
            n2ml=n2ml,
            mbv=mbv,
        )

    consts = dict(
        iota512=np.tile(np.arange(W, dtype=np.float32), (P, 1)),
        iota512b=np.tile(np.arange(W, 2 * W, dtype=np.float32), (P, 1)),
        iota128=np.tile(np.arange(P, dtype=np.float32), (P, 1)),
        iota64=np.tile(np.arange(B, dtype=np.float32), (P, 1)),
        ident=np.eye(P, dtype=np.float32),
        gcw1=np.concatenate([np.asarray(inputs["gc_W1"][i], np.float32) for i in range(NGC)], 1),
        gcw2=np.concatenate([np.asarray(inputs["gc_W2"][i], np.float32) for i in range(NGC)], 1),
        gcb1=np.stack([np.asarray(inputs["gc_b1"][i], np.float32) for i in range(NGC)], 1),
        gcb2=np.stack([np.asarray(inputs["gc_b2"][i], np.float32) for i in range(NGC)], 1),
        mcw1=np.concatenate([np.asarray(inputs["mc_W1"][i], np.float32) for i in range(NMC)], 1),
        mcw2=np.concatenate([np.asarray(inputs["mc_W2"][i], np.float32) for i in range(NMC)], 1),
        mcb1=np.stack([np.asarray(inputs["mc_b1"][i], np.float32) for i in range(NMC)], 1),
        mcb2=np.stack([np.asarray(inputs["mc_b2"][i], np.float32) for i in range(NMC)], 1),
        linw=np.concatenate(
            [np.asarray(inputs["lin_W"], np.float32)[k * D : (k + 1) * D, :] for k in range(NGC)],
            axis=1,
        ),
        linb=np.asarray(inputs["lin_b"], np.float32).reshape(D, 1),
        x_rep=x_rep,
    )
    geom = dict(
        S_pad=S_pad, NW=NW, NT=NT, NG=NG, H=H, sched=sched, T_pad=T_pad,
        M_pad=M_pad, MW=MW, MG=MG, msched=msched, MT_pad=MT_pad,
    )
    return consts, per_core, geom


def _build(consts, geom):
    S_pad, NW, NT, NG, H = geom["S_pad"], geom["NW"], geom["NT"], geom["NG"], geom["H"]
    sched, T_pad = geom["sched"], geom["T_pad"]
    M_pad, MW, MG = geom["M_pad"], geom["MW"], geom["MG"]
    msched, MT_pad = geom["msched"], geom["MT_pad"]

    nc = bass.Bass(num_devices=NCORES)
    dp = lambda n, s, dt=FP: nc.declare_dram_parameter(n, list(s), dt, isOutput=False)

    ins = {}
    for n, a in consts.items():
        ins[n] = dp(n, a.shape)
    eidx_d = dp("eidx", [P, T_pad], mybir.dt.int32)
    edst_d = dp("edst", [P, T_pad])
    eap_d = dp("eap", [P, T_pad * D])
    batchv_d = dp("batchv", [P, NT])
    xsrc0_d = dp("xsrc0", [P, T_pad * D])
    x0T_d = dp("x0T", [D, S_pad])
    midx_d = dp("midx", [P, MT_pad], mybir.dt.int32)
    mdst_d = dp("mdst", [P, MT_pad])
    membT_d = dp("membT", [D, M_pad])
    n2ml_d = dp("n2ml", [P, NT])
    mbv_d = dp("mbv", [P, MW])

    xg_o = nc.declare_dram_parameter("xg_part", [B, NGC * D], FP, isOutput=True)
    xm_o = nc.declare_dram_parameter("xm_part", [B, NMC * D], FP, isOutput=True)

    ag = [nc.dram_tensor(f"ag{i}", [NG, D], FP, addr_space="Shared") for i in range(2)]
    xr = [nc.dram_tensor(f"xr{i}", [S_pad, D], FP) for i in range(2)]
    xt = [nc.dram_tensor(f"xt{i}", [D, S_pad], FP) for i in range(NGC)]
    mrows = [nc.dram_tensor(f"mrows{i}", [M_pad, D], FP) for i in range(2)]
    agm = [nc.dram_tensor(f"agm{i}", [MG, D], FP, addr_space="Shared") for i in range(2)]

    RG = [list(range(NCORES))]

    regcache = {}

    def nidx_reg(v):
        if v not in regcache:
            regcache[v] = nc.gpsimd.to_reg(v)
        return regcache[v]

    with TileContext(nc) as tc:
        with (
            tc.tile_pool(name="const", bufs=1) as cp,
            tc.tile_pool(name="gat", bufs=3) as gp,
            tc.tile_pool(name="eat", bufs=3) as ep,
            tc.tile_pool(name="sel", bufs=4) as sp,
            tc.tile_pool(name="win", bufs=3) as wp,
            tc.tile_pool(name="rows", bufs=3) as rp,
            tc.tile_pool(name="acc", bufs=1) as ap,
            tc.tile_pool(name="pagg", bufs=2, space="PSUM") as pagg_p,
            tc.tile_pool(name="pmlp", bufs=2, space="PSUM") as pmlp_p,
            tc.tile_pool(name="ptr", bufs=2, space="PSUM") as ptr_p,
            tc.tile_pool(name="pacc", bufs=1, space="PSUM") as pacc_p,
        ):
            # resident constants
            C = {}
            for n, a in consts.items():
                if n == "x_rep":
                    continue
                t = cp.tile(list(a.shape), FP, tag=n)
                nc.sync.dma_start(out=t[:], in_=ins[n][:])
                C[n] = t
            eidx_t = cp.tile([P, T_pad], mybir.dt.int32, tag="eidx")
            nc.sync.dma_start(out=eidx_t[:], in_=eidx_d[:])
            edst_t = cp.tile([P, T_pad], FP, tag="edst")
            nc.sync.dma_start(out=edst_t[:], in_=edst_d[:])
            batchv_t = cp.tile([P, NT], FP, tag="batchv")
            nc.sync.dma_start(out=batchv_t[:], in_=batchv_d[:])
            n2ml_t = cp.tile([P, NT], FP, tag="n2ml")
            nc.sync.dma_start(out=n2ml_t[:], in_=n2ml_d[:])
            midx_t = cp.tile([P, MT_pad], mybir.dt.int32, tag="midx")
            nc.sync.dma_start(out=midx_t[:], in_=midx_d[:])
            mdst_t = cp.tile([P, MT_pad], FP, tag="mdst")
            nc.sync.dma_start(out=mdst_t[:], in_=mdst_d[:])
            mbv_t = cp.tile([P, MW], FP, tag="mbv")
            nc.sync.dma_start(out=mbv_t[:], in_=mbv_d[:])
            membT_t = cp.tile([D, M_pad], FP, tag="membT")
            nc.sync.dma_start(out=membT_t[:], in_=membT_d[:])

            xg_sb = ap.tile([B, NGC * D], FP, tag="xg")
            xm_sb = ap.tile([B, NMC * D], FP, tag="xm")

            # 128-window -> list of (tpos, nt) runs; chunked to <= G tiles
            win_chunks = [[] for _ in range(NT)]
            tpos = 0
            for (w, nt) in sched:
                off = 0
                while off < nt:
                    n = min(G, nt - off)
                    win_chunks[w].append((tpos + off, n))
                    off += n
                tpos += nt
            win_tiles = [sum(c[1] for c in win_chunks[w]) for w in range(NT)]

            # ---------------- Phase A: 3 GINE layers ----------------
            for i in range(NGC):
                src_d = ins["x_rep"] if i == 0 else ag[i - 1]
                prevT = x0T_d if i == 0 else xt[i - 1]
                pxg = pacc_p.tile([B, D], FP, tag="acc")
                for w in range(NW):
                    xw = wp.tile([P, W], FP, tag="xw")
                    nc.sync.dma_start(out=xw[:], in_=prevT[:, w * W : (w + 1) * W])
                    tT = wp.tile([P, W], FP, tag="tT")
                    for sub in range(4):
                      w128 = w * 4 + sub
                      ntw = win_tiles[w128]
                      pagg = pagg_p.tile([P, P], FP, tag="agg")
                      ti = 0
                      for (t0, nt) in win_chunks[w128]:
                        eat = ep.tile([P, G * D], FP, tag="eat")
                        nc.sync.dma_start(
                            out=eat[:, : nt * D], in_=eap_d[:, t0 * D : (t0 + nt) * D]
                        )
                        if i == 0:
                            gx0 = gp.tile([P, G * D], FP, tag="gx0")
                            nc.sync.dma_start(
                                out=gx0[:, : nt * D],
                                in_=xsrc0_d[:, t0 * D : (t0 + nt) * D],
                            )
                            nc.vector.tensor_add(
                                out=eat[:, : nt * D], in0=eat[:, : nt * D],
                                in1=gx0[:, : nt * D],
                            )
                            nc.scalar.activation(
                                out=eat[:, : nt * D], in_=eat[:, : nt * D],
                                func=mybir.ActivationFunctionType.Relu,
                            )
                        else:
                            for t in range(nt):
                                gx = gp.tile([P, D], FP, tag="gx")
                                nc.gpsimd.indirect_dma_start(
                                    out=gx[:], out_offset=None, in_=src_d[:],
                                    in_offset=bass.IndirectOffsetOnAxis(
                                        ap=eidx_t[:, t0 + t : t0 + t + 1], axis=0
                                    ),
                                )
                                nc.vector.tensor_add(
                                    out=eat[:, t * D : (t + 1) * D],
                                    in0=eat[:, t * D : (t + 1) * D], in1=gx[:],
                                )
                                nc.scalar.activation(
                                    out=eat[:, t * D : (t + 1) * D],
                                    in_=eat[:, t * D : (t + 1) * D],
                                    func=mybir.ActivationFunctionType.Relu,
                                )
                        for t in range(nt):
                            S = sp.tile([P, P], FP, tag="S")
                            nc.vector.tensor_tensor(
                                out=S[:],
                                in0=edst_t[:, t0 + t : t0 + t + 1].to_broadcast([P, P]),
                                in1=C["iota128"][:],
                                op=mybir.AluOpType.is_equal,
                            )
                            nc.tensor.matmul(
                                out=pagg[:], lhsT=eat[:, t * D : (t + 1) * D], rhs=S[:],
                                start=(ti == 0), stop=(ti == ntw - 1),
                            )
                            ti += 1
                      if ntw:
                        nc.vector.tensor_add(
                            out=tT[:, sub * P : (sub + 1) * P],
                            in0=xw[:, sub * P : (sub + 1) * P], in1=pagg[:],
                        )
                      else:
                        nc.vector.tensor_copy(
                            out=tT[:, sub * P : (sub + 1) * P],
                            in_=xw[:, sub * P : (sub + 1) * P],
                        )
                    p1 = pmlp_p.tile([P, W], FP, tag="pm")
                    nc.tensor.matmul(
                        out=p1[:], lhsT=C["gcw1"][:, i * D : (i + 1) * D], rhs=tT[:],
                        start=True, stop=True,
                    )
                    h1 = wp.tile([P, W], FP, tag="h1")
                    nc.scalar.activation(
                        out=h1[:], in_=p1[:], func=mybir.ActivationFunctionType.Relu,
                        bias=C["gcb1"][:, i : i + 1],
                    )
                    p2 = pmlp_p.tile([P, W], FP, tag="pm")
                    nc.tensor.matmul(
                        out=p2[:], lhsT=C["gcw2"][:, i * D : (i + 1) * D], rhs=h1[:],
                        start=True, stop=True,
                    )
                    xnT = wp.tile([P, W], FP, tag="xnT")
                    nc.scalar.activation(
                        out=xnT[:], in_=p2[:], func=mybir.ActivationFunctionType.Relu,
                        bias=C["gcb2"][:, i : i + 1],
                    )
                    nc.sync.dma_start(out=xt[i][:, w * W : (w + 1) * W], in_=xnT[:])
                    for q in range(4):
                        pt = ptr_p.tile([P, P], FP, tag="pt")
                        nc.tensor.transpose(
                            out=pt[:], in_=xnT[:, q * P : (q + 1) * P], identity=C["ident"][:]
                        )
                        rows = rp.tile([P, P], FP, tag="rows")
                        nc.vector.tensor_copy(out=rows[:], in_=pt[:])
                        if i < 2:
                            nc.sync.dma_start(
                                out=xr[i][w * W + q * P : w * W + (q + 1) * P, :],
                                in_=rows[:],
                            )
                        col = w * 4 + q
                        Sb = sp.tile([P, B], FP, tag="Sb")
                        nc.vector.tensor_tensor(
                            out=Sb[:],
                            in0=batchv_t[:, col : col + 1].to_broadcast([P, B]),
                            in1=C["iota64"][:],
                            op=mybir.AluOpType.is_equal,
                        )
                        nc.tensor.matmul(
                            out=pxg[:], lhsT=Sb[:], rhs=rows[:],
                            start=(col == 0), stop=(col == NT - 1),
                        )
                nc.vector.tensor_copy(out=xg_sb[:, i * D : (i + 1) * D], in_=pxg[:])
                if i < 2:
                    nc.gpsimd.collective_compute(
                        "AllGather", mybir.AluOpType.bypass, replica_groups=RG,
                        ins=[xr[i][:]], outs=[ag[i][:]],
                    )
            nc.sync.dma_start(out=xg_o[:], in_=xg_sb[:])

            # ---------------- Phase C: motif features ----------------
            pxmh = [pacc_p.tile([P, W], FP, tag="acc" if mw == 0 else "acc2", name=f"pxmh{mw}") for mw in range(2)]
            for w in range(NW):
                pC = pmlp_p.tile([P, W], FP, tag="pm")
                for k in range(NGC):
                    xk = wp.tile([P, W], FP, tag="xw")
                    nc.sync.dma_start(out=xk[:], in_=xt[k][:, w * W : (w + 1) * W])
                    nc.tensor.matmul(
                        out=pC[:], lhsT=C["linw"][:, k * D : (k + 1) * D], rhs=xk[:],
                        start=(k == 0), stop=(k == NGC - 1),
                    )
                xmnT = wp.tile([P, W], FP, tag="h1")
                nc.scalar.activation(
                    out=xmnT[:], in_=pC[:], func=mybir.ActivationFunctionType.Relu,
                    bias=C["linb"][:],
                )
                for q in range(4):
                    pt = ptr_p.tile([P, P], FP, tag="pt")
                    nc.tensor.transpose(
                        out=pt[:], in_=xmnT[:, q * P : (q + 1) * P], identity=C["ident"][:]
                    )
                    rows = rp.tile([P, P], FP, tag="rows")
                    nc.vector.tensor_copy(out=rows[:], in_=pt[:])
                    col = w * 4 + q
                    for mw in range(2):
                        Sm = sp.tile([P, W], FP, tag="S")
                        nc.vector.tensor_tensor(
                            out=Sm[:],
                            in0=n2ml_t[:, col : col + 1].to_broadcast([P, W]),
                            in1=C["iota512" if mw == 0 else "iota512b"][:],
                            op=mybir.AluOpType.is_equal,
                        )
                        nc.tensor.matmul(
                            out=pxmh[mw][:], lhsT=rows[:], rhs=Sm[:],
                            start=(col == 0), stop=(col == NT - 1),
                        )
            xmhT = ap.tile([D, M_pad], FP, tag="xmhT")
            nc.vector.tensor_add(
                out=xmhT[:, :W], in0=pxmh[0][:], in1=membT_t[:, :W]
            )
            if M_pad > W:
                nc.vector.tensor_add(
                    out=xmhT[:, W:M_pad], in0=pxmh[1][:, : M_pad - W],
                    in1=membT_t[:, W:M_pad],
                )
            for mw in range(MW):
                pt = ptr_p.tile([P, P], FP, tag="pt")
                nc.tensor.transpose(
                    out=pt[:], in_=xmhT[:, mw * P : (mw + 1) * P], identity=C["ident"][:]
                )
                rows = rp.tile([P, P], FP, tag="rows")
                nc.vector.tensor_copy(out=rows[:], in_=pt[:])
                nc.sync.dma_start(out=mrows[0][mw * P : (mw + 1) * P, :], in_=rows[:])
            nc.gpsimd.collective_compute(
                "AllGather", mybir.AluOpType.bypass, replica_groups=RG,
                ins=[mrows[0][:]], outs=[agm[0][:]],
            )

            # ---------------- motif GIN layers ----------------
            mwin_chunks = [[] for _ in range(MW)]
            tpos = 0
            for (w, nt) in msched:
                off = 0
                while off < nt:
                    n = min(G, nt - off)
                    mwin_chunks[w].append((tpos + off, n))
                    off += n
                tpos += nt
            mwin_tiles = [sum(c[1] for c in mwin_chunks[w]) for w in range(MW)]

            hT_prev = xmhT
            for i in range(NMC):
                pxm = pacc_p.tile([B, D], FP, tag="acc")
                hT_new = ap.tile([D, M_pad], FP, tag=f"mh{i}T")
                for w in range(MW):
                    ntw = mwin_tiles[w]
                    pagg = pagg_p.tile([P, P], FP, tag="agg")
                    ti = 0
                    for (t0, nt) in mwin_chunks[w]:
                        gm = gp.tile([P, G * D], FP, tag="gx")
                        for t in range(nt):
                            nc.gpsimd.indirect_dma_start(
                                out=gm[:, t * D : (t + 1) * D], out_offset=None,
                                in_=agm[i][:],
                                in_offset=bass.IndirectOffsetOnAxis(
                                    ap=midx_t[:, t0 + t : t0 + t + 1], axis=0
                                ),
                            )
                        for t in range(nt):
                            Sm = sp.tile([P, P], FP, tag="Sm128")
                            nc.vector.tensor_tensor(
                                out=Sm[:],
                                in0=mdst_t[:, t0 + t : t0 + t + 1].to_broadcast([P, P]),
                                in1=C["iota128"][:],
                                op=mybir.AluOpType.is_equal,
                            )
                            nc.tensor.matmul(
                                out=pagg[:], lhsT=gm[:, t * D : (t + 1) * D], rhs=Sm[:],
                                start=(ti == 0), stop=(ti == ntw - 1),
                            )
                            ti += 1
                    tT = wp.tile([P, P], FP, tag="mtT")
                    if ntw:
                        nc.vector.tensor_add(
                            out=tT[:], in0=hT_prev[:, w * P : (w + 1) * P], in1=pagg[:]
                        )
                    else:
                        nc.vector.tensor_copy(out=tT[:], in_=hT_prev[:, w * P : (w + 1) * P])
                    p1 = pmlp_p.tile([P, P], FP, tag="pm")
                    nc.tensor.matmul(
                        out=p1[:], lhsT=C["mcw1"][:, i * D : (i + 1) * D], rhs=tT[:],
                        start=True, stop=True,
                    )
                    h1 = wp.tile([P, P], FP, tag="mh1")
                    nc.scalar.activation(
                        out=h1[:], in_=p1[:], func=mybir.ActivationFunctionType.Relu,
                        bias=C["mcb1"][:, i : i + 1],
                    )
                    p2 = pmlp_p.tile([P, P], FP, tag="pm")
                    nc.tensor.matmul(
                        out=p2[:], lhsT=C["mcw2"][:, i * D : (i + 1) * D], rhs=h1[:],
                        start=True, stop=True,
                    )
                    nc.scalar.activation(
                        out=hT_new[:, w * P : (w + 1) * P], in_=p2[:],
                        func=mybir.ActivationFunctionType.Relu,
                        bias=C["mcb2"][:, i : i + 1],
                    )
                    pt = ptr_p.tile([P, P], FP, tag="pt")
                    nc.tensor.transpose(
                        out=pt[:], in_=hT_new[:, w * P : (w + 1) * P],
                        identity=C["ident"][:],
                    )
                    rows = rp.tile([P, P], FP, tag="rows")
                    nc.vector.tensor_copy(out=rows[:], in_=pt[:])
                    if i == 0:
                        nc.sync.dma_start(
                            out=mrows[1][w * P : (w + 1) * P, :], in_=rows[:]
                        )
                    Smb = sp.tile([P, B], FP, tag="Sb")
                    nc.vector.tensor_tensor(
                        out=Smb[:],
                        in0=mbv_t[:, w : w + 1].to_broadcast([P, B]),
                        in1=C["iota64"][:],
                        op=mybir.AluOpType.is_equal,
                    )
                    nc.tensor.matmul(
                        out=pxm[:], lhsT=Smb[:], rhs=rows[:],
                        start=(w == 0), stop=(w == MW - 1),
                    )
                nc.vector.tensor_copy(out=xm_sb[:, i * D : (i + 1) * D], in_=pxm[:])
                if i == 0:
                    nc.gpsimd.collective_compute(
                        "AllGather", mybir.AluOpType.bypass, replica_groups=RG,
                        ins=[mrows[1][:]], outs=[agm[1][:]],
                    )
                hT_prev = hT_new
            nc.sync.dma_start(out=xm_o[:], in_=xm_sb[:])

    _split_multiwait(nc)
    return nc


def kernel(**inputs):
    consts, per_core, geom = _prep(inputs)
    nc = _build(consts, geom)
    in_maps = []
    for c in range(NCORES):
        m = dict(consts)
        m.update(per_core[c])
        in_maps.append({k: np.ascontiguousarray(v) for k, v in m.items()})
    res = run_bass_kernel_spmd(nc, in_maps, list(range(NCORES)))
    xg = np.zeros((B, NGC * D), np.float32)
    xm = np.zeros((B, NMC * D), np.float32)
    for r in res.results:
        xg += r["xg_part"]
        xm += r["xm_part"]
    kernel._last_exec_ns = res.exec_time_ns
    return xm, xg


# revision 15
# speedup vs baseline: 1.0515x; 1.0165x over previous
"""MPGIN encoder distributed across 8 TRN2 NeuronCores.

Strategy (graph-partition per sharding hint):
 - Nodes sharded batch-aligned: core c owns batches [8c, 8c+8) -> contiguous
   node range (batch is sorted). Edges sharded by dst owner, sorted by
   (dst-window, src-half), padded to a common SPMD schedule.
 - segment_sum via one-hot matmul: for each 512-node window, accumulate
   msg.T @ S into PSUM where S[e, n] = (dstoff[e] == n) built on DVE.
 - Activations kept transposed [dim, nodes] so the GIN MLPs run with
   stationary weights; PE transposes recover row-major for gathers/readouts.
 - x[src] gathers via gpsimd dma_gather (int16 idx) from a replicated
   row-major buffer refreshed per layer by AllGather.
 - Graph/motif readouts via one-hot matmul vs batch ids; per-core partials
   summed on host.
"""
import numpy as np

import concourse.bass as bass
import concourse.mybir as mybir
from concourse.bass_utils import run_bass_kernel_spmd
from concourse.tile import TileContext

P = 128
W = 512            # node window for agg/MLP
G = 16             # gather chunk size in tiles
NCORES = 8
D = 128
NGC, NMC = 3, 2
B = 64
FP = mybir.dt.float32
I16 = mybir.dt.int16


# ---------------------------------------------------------------------------
# walrus workaround: split instructions carrying >1 semaphore waits
def _split_multiwait(nc, max_waits=1):
    ctr = [0]
    f = nc.m.functions[0]
    for bb in f.blocks:
        insts = list(bb.instructions)
        out = []
        changed = False
        for inst in insts:
            si = getattr(inst, "sync_info", None)
            waits = list(si.on_wait) if si is not None else []
            if len(waits) > max_waits:
                keep = waits[-max_waits:]
                for w in waits[:-max_waits]:
                    ctr[0] += 1
                    ev = mybir.InstEventSemaphore(
                        name=f"WSPLIT-{ctr[0]}", engine=inst.engine, ins=[], outs=[]
                    )
                    ev.sync_info = mybir.SyncInfo(on_wait=[w], on_update=[])
                    out.append(ev)
                si.on_wait = keep
                changed = True
            out.append(inst)
        if changed:
            bb.instructions = out


def _ceil(a, b):
    return -(-a // b)


def _pack_idx16(flat):
    """Pack flat int indices into dma_gather idx layout [128, len//16] int16."""
    n = len(flat)
    assert n % 128 == 0
    ef = np.asarray(flat, np.int16).reshape(n // 128, 8, 16)
    idx16 = np.zeros((16, n // 16), np.int16)
    for p in range(16):
        idx16[p] = ef[:, :, p].reshape(-1)
    return np.tile(idx16, (8, 1))


def _pack_cols(vals, ncols, fill):
    """Pack per-slot values [ntiles*128] into [128, ntiles] column layout."""
    v = np.full(ncols * 128, fill, np.float32)
    v[: len(vals)] = vals
    return v.reshape(ncols, 128).T.copy()


def _prep(inputs):
    x = np.asarray(inputs["x"], np.float32)
    ea = np.asarray(inputs["edge_attr"], np.float32)
    eidx = np.asarray(inputs["edge_index"], np.int64)
    batch = np.asarray(inputs["batch"], np.int64)
    node2motif = np.asarray(inputs["node2motif"], np.int64)
    num_motifs = np.asarray(inputs["num_motifs"], np.int64)
    meidx = np.asarray(inputs["motif_edge_index"], np.int64)
    motifid = np.asarray(inputs["motifid"], np.int64)
    emb = np.asarray(inputs["emb"], np.float32)
    N = x.shape[0]
    E = eidx.shape[1]
    M_TOT = int(num_motifs.sum())

    # ---- node shards: batches [8c, 8c+8) ----
    bcounts = np.bincount(batch, minlength=B)
    bstart = np.concatenate([[0], np.cumsum(bcounts)])
    nstart = np.array([bstart[8 * c] for c in range(NCORES)] + [N])
    slen = nstart[1:] - nstart[:-1]
    S_pad = int(_ceil(max(slen.max(), 1), W) * W)
    NW = S_pad // W
    NT = S_pad // P
    NG = NCORES * S_pad
    H = NG // 2
    assert H < 32768

    owner_of_node = np.searchsorted(nstart[1:], np.arange(N), side="right")
    gidx_of_node = owner_of_node * S_pad + (np.arange(N) - nstart[owner_of_node])

    # ---- edges ----
    src, dst = eidx[0], eidx[1]
    eown = owner_of_node[dst]
    ldst = dst - nstart[eown]
    ewin = ldst // P
    esrc_g = gidx_of_node[src]

    # counts[c, w] over 128-node windows
    counts = np.zeros((NCORES, NT), np.int64)
    np.add.at(counts, (eown, ewin), 1)
    ntile = _ceil(counts, P).max(axis=0)  # [NT] common schedule
    sched = [(w, int(ntile[w])) for w in range(NT) if ntile[w]]
    T_pad = sum(s[1] for s in sched)

    order = np.lexsort((ewin, eown))
    so_src = esrc_g[order]
    so_ldst = ldst[order]
    so_ea = order  # ea row ids
    cs = np.concatenate([[0], np.cumsum(counts.reshape(-1))])
    cstart = cs[:-1].reshape(NCORES, NT)

    per_core = []
    for c in range(NCORES):
        idxflat = np.zeros(T_pad * P, np.int64)
        dstflat = np.full(T_pad * P, -1.0, np.float32)
        earow = np.full(T_pad * P, -1, np.int64)
        tpos = 0
        for (w, nt) in sched:
            cnt = int(counts[c, w])
            s0 = int(cstart[c, w])
            sl = slice(tpos * P, tpos * P + cnt)
            idxflat[sl] = so_src[s0 : s0 + cnt]
            dstflat[sl] = (so_ldst[s0 : s0 + cnt] % P).astype(np.float32)
            earow[sl] = so_ea[s0 : s0 + cnt]
            tpos += nt
        ea_pack = np.zeros((T_pad * P, D), np.float32)
        valid = earow >= 0
        ea_pack[valid] = ea[earow[valid]]
        ea_pack = ea_pack.reshape(T_pad, P, D).transpose(1, 0, 2).reshape(P, T_pad * D)
        eidx32 = idxflat.reshape(T_pad, P).T.astype(np.int32).copy()
        edstp = dstflat.reshape(T_pad, P).T.copy()

        lc = int(slen[c])
        batchv = _pack_cols(batch[nstart[c] : nstart[c] + lc].astype(np.float32), NT, -1.0)
        per_core.append(
            dict(eidx=eidx32, edst=edstp, eap=ea_pack, batchv=batchv,
                 _idxflat=idxflat)
        )

    # replicated padded x0 rows + per-core transposed shard
    x_rep = np.zeros((NG, D), np.float32)
    x_rep[gidx_of_node] = x
    for c in range(NCORES):
        x0T = np.zeros((D, S_pad), np.float32)
        lc = int(slen[c])
        x0T[:, :lc] = x[nstart[c] : nstart[c] + lc].T
        per_core[c]["x0T"] = x0T
        # layer-0 halo: pre-gathered x0[src] stream in edge-slot order
        idxflat = per_core[c].pop("_idxflat")
        xs0 = x_rep[idxflat]
        per_core[c]["xsrc0"] = (
            xs0.reshape(T_pad, P, D).transpose(1, 0, 2).reshape(P, T_pad * D)
        )

    # ---- motifs ----
    partial = np.concatenate([[0], np.cumsum(num_motifs)[:-1]])
    n2m = node2motif + partial[batch]
    mstart = np.array([int(partial[8 * c]) for c in range(NCORES)] + [M_TOT])
    mslen = mstart[1:] - mstart[:-1]
    M_pad = int(_ceil(max(mslen.max(), 1), P) * P)
    MW = M_pad // P
    MG = NCORES * M_pad
    assert MG < 32768
    owner_of_motif = np.searchsorted(mstart[1:], np.arange(M_TOT), side="right")
    gidx_of_motif = owner_of_motif * M_pad + (np.arange(M_TOT) - mstart[owner_of_motif])
    motif_batch = np.searchsorted(np.cumsum(num_motifs), np.arange(M_TOT), side="right")

    msrc, mdst = meidx[0], meidx[1]
    mown = owner_of_motif[mdst]
    mldst = mdst - mstart[mown]
    mwin = mldst // P
    msrc_g = gidx_of_motif[msrc]
    mcounts = np.zeros((NCORES, MW), np.int64)
    np.add.at(mcounts, (mown, mwin), 1)
    mntile = _ceil(mcounts, P).max(axis=0)  # [MW]
    msched = [(w, int(mntile[w])) for w in range(MW) if mntile[w]]
    MT_pad = sum(s[1] for s in msched)

    morder = np.lexsort((mwin, mown))
    mo_src = msrc_g[morder]
    mo_ldst = mldst[morder]
    mcs = np.concatenate([[0], np.cumsum(mcounts.reshape(-1))])
    mcstart = mcs[:-1].reshape(NCORES, MW)

    embrows = emb[motifid]  # [M_TOT, D]
    for c in range(NCORES):
        midxflat = np.zeros(MT_pad * P, np.int64)
        mdstflat = np.full(MT_pad * P, -1.0, np.float32)
        tpos = 0
        for (w, nt) in msched:
            cnt = int(mcounts[c, w])
            s0 = int(mcstart[c, w])
            sl = slice(tpos * P, tpos * P + cnt)
            midxflat[sl] = mo_src[s0 : s0 + cnt]
            mdstflat[sl] = (mo_ldst[s0 : s0 + cnt] % P).astype(np.float32)
            tpos += nt
        lm = int(mslen[c])
        membT = np.zeros((D, M_pad), np.float32)
        membT[:, :lm] = embrows[mstart[c] : mstart[c] + lm].T
        n2ml = _pack_cols(
            (n2m[nstart[c] : nstart[c] + int(slen[c])] - mstart[c]).astype(np.float32),
            NT, -1.0,
        )
        mbv = _pack_cols(
            motif_batch[mstart[c] : mstart[c] + lm].astype(np.float32), MW, -1.0
        )
        per_core[c].update(
            midx=midxflat.reshape(MT_pad, P).T.astype(np.int32).copy(),
            mdst=mdstflat.reshape(MT_pad, P).T.copy(),
            membT=membT,
            n2ml=n2ml,
            mbv=mbv,
        )

    consts = dict(
        iota512=np.tile(np.arange(W, dtype=np.float32), (P, 1)),
        iota512b=np.tile(np.arange(W, 2 * W, dtype=np.float32), (P, 1)),
        iota128=np.tile(np.arange(P, dtype=np.float32), (P, 1)),
        iota64=np.tile(np.arange(B, dtype=np.float32), (P, 1)),
        ident=np.eye(P, dtype=np.float32),
        gcw1=np.concatenate([np.asarray(inputs["gc_W1"][i], np.float32) for i in range(NGC)], 1),
        gcw2=np.concatenate([np.asarray(inputs["gc_W2"][i], np.float32) for i in range(NGC)], 1),
        gcb1=np.stack([np.asarray(inputs["gc_b1"][i], np.float32) for i in range(NGC)], 1),
        gcb2=np.stack([np.asarray(inputs["gc_b2"][i], np.float32) for i in range(NGC)], 1),
        mcw1=np.concatenate([np.asarray(inputs["mc_W1"][i], np.float32) for i in range(NMC)], 1),
        mcw2=np.concatenate([np.asarray(inputs["mc_W2"][i], np.float32) for i in range(NMC)], 1),
        mcb1=np.stack([np.asarray(inputs["mc_b1"][i], np.float32) for i in range(NMC)], 1),
        mcb2=np.stack([np.asarray(inputs["mc_b2"][i], np.float32) for i in range(NMC)], 1),
        linw=np.concatenate(
            [np.asarray(inputs["lin_W"], np.float32)[k * D : (k + 1) * D, :] for k in range(NGC)],
            axis=1,
        ),
        linb=np.asarray(inputs["lin_b"], np.float32).reshape(D, 1),
        x_rep=x_rep,
    )
    geom = dict(
        S_pad=S_pad, NW=NW, NT=NT, NG=NG, H=H, sched=sched, T_pad=T_pad,
        M_pad=M_pad, MW=MW, MG=MG, msched=msched, MT_pad=MT_pad,
    )
    return consts, per_core, geom


def _build(consts, geom):
    S_pad, NW, NT, NG, H = geom["S_pad"], geom["NW"], geom["NT"], geom["NG"], geom["H"]
    sched, T_pad = geom["sched"], geom["T_pad"]
    M_pad, MW, MG = geom["M_pad"], geom["MW"], geom["MG"]
    msched, MT_pad = geom["msched"], geom["MT_pad"]

    nc = bass.Bass(num_devices=NCORES)
    dp = lambda n, s, dt=FP: nc.declare_dram_parameter(n, list(s), dt, isOutput=False)

    ins = {}
    for n, a in consts.items():
        ins[n] = dp(n, a.shape)
    eidx_d = dp("eidx", [P, T_pad], mybir.dt.int32)
    edst_d = dp("edst", [P, T_pad])
    eap_d = dp("eap", [P, T_pad * D])
    batchv_d = dp("batchv", [P, NT])
    xsrc0_d = dp("xsrc0", [P, T_pad * D])
    x0T_d = dp("x0T", [D, S_pad])
    midx_d = dp("midx", [P, MT_pad], mybir.dt.int32)
    mdst_d = dp("mdst", [P, MT_pad])
    membT_d = dp("membT", [D, M_pad])
    n2ml_d = dp("n2ml", [P, NT])
    mbv_d = dp("mbv", [P, MW])

    xg_o = nc.declare_dram_parameter("xg_part", [B, NGC * D], FP, isOutput=True)
    xm_o = nc.declare_dram_parameter("xm_part", [B, NMC * D], FP, isOutput=True)

    ag = [nc.dram_tensor(f"ag{i}", [NG, D], FP, addr_space="Shared") for i in range(2)]
    xr = [nc.dram_tensor(f"xr{i}", [S_pad, D], FP) for i in range(2)]
    xt = [nc.dram_tensor(f"xt{i}", [D, S_pad], FP) for i in range(NGC)]
    mrows = [nc.dram_tensor(f"mrows{i}", [M_pad, D], FP) for i in range(2)]
    agm = [nc.dram_tensor(f"agm{i}", [MG, D], FP, addr_space="Shared") for i in range(2)]

    RG = [list(range(NCORES))]

    regcache = {}

    def nidx_reg(v):
        if v not in regcache:
            regcache[v] = nc.gpsimd.to_reg(v)
        return regcache[v]

    with TileContext(nc) as tc:
        with (
            tc.tile_pool(name="const", bufs=1) as cp,
            tc.tile_pool(name="gat", bufs=3) as gp,
            tc.tile_pool(name="eat", bufs=4) as ep,
            tc.tile_pool(name="sel", bufs=8) as sp,
            tc.tile_pool(name="win", bufs=3) as wp,
            tc.tile_pool(name="rows", bufs=3) as rp,
            tc.tile_pool(name="acc", bufs=1) as ap,
            tc.tile_pool(name="pagg", bufs=2, space="PSUM") as pagg_p,
            tc.tile_pool(name="pmlp", bufs=2, space="PSUM") as pmlp_p,
            tc.tile_pool(name="ptr", bufs=2, space="PSUM") as ptr_p,
            tc.tile_pool(name="pacc", bufs=1, space="PSUM") as pacc_p,
        ):
            # resident constants
            C = {}
            for n, a in consts.items():
                if n == "x_rep":
                    continue
                t = cp.tile(list(a.shape), FP, tag=n)
                nc.sync.dma_start(out=t[:], in_=ins[n][:])
                C[n] = t
            eidx_t = cp.tile([P, T_pad], mybir.dt.int32, tag="eidx")
            nc.sync.dma_start(out=eidx_t[:], in_=eidx_d[:])
            edst_t = cp.tile([P, T_pad], FP, tag="edst")
            nc.sync.dma_start(out=edst_t[:], in_=edst_d[:])
            batchv_t = cp.tile([P, NT], FP, tag="batchv")
            nc.sync.dma_start(out=batchv_t[:], in_=batchv_d[:])
            n2ml_t = cp.tile([P, NT], FP, tag="n2ml")
            nc.sync.dma_start(out=n2ml_t[:], in_=n2ml_d[:])
            midx_t = cp.tile([P, MT_pad], mybir.dt.int32, tag="midx")
            nc.sync.dma_start(out=midx_t[:], in_=midx_d[:])
            mdst_t = cp.tile([P, MT_pad], FP, tag="mdst")
            nc.sync.dma_start(out=mdst_t[:], in_=mdst_d[:])
            mbv_t = cp.tile([P, MW], FP, tag="mbv")
            nc.sync.dma_start(out=mbv_t[:], in_=mbv_d[:])
            membT_t = cp.tile([D, M_pad], FP, tag="membT")
            nc.sync.dma_start(out=membT_t[:], in_=membT_d[:])

            xg_sb = ap.tile([B, NGC * D], FP, tag="xg")
            xm_sb = ap.tile([B, NMC * D], FP, tag="xm")

            # 128-window -> list of (tpos, nt) runs; chunked to <= G tiles
            win_chunks = [[] for _ in range(NT)]
            tpos = 0
            for (w, nt) in sched:
                off = 0
                while off < nt:
                    n = min(G, nt - off)
                    win_chunks[w].append((tpos + off, n))
                    off += n
                tpos += nt
            win_tiles = [sum(c[1] for c in win_chunks[w]) for w in range(NT)]

            # ---------------- Phase A: 3 GINE layers ----------------
            for i in range(NGC):
                src_d = ins["x_rep"] if i == 0 else ag[i - 1]
                prevT = x0T_d if i == 0 else xt[i - 1]
                pxg = pacc_p.tile([B, D], FP, tag="acc")
                for w in range(NW):
                    xw = wp.tile([P, W], FP, tag="xw")
                    nc.sync.dma_start(out=xw[:], in_=prevT[:, w * W : (w + 1) * W])
                    tT = wp.tile([P, W], FP, tag="tT")
                    for sub in range(4):
                      w128 = w * 4 + sub
                      ntw = win_tiles[w128]
                      pagg = pagg_p.tile([P, P], FP, tag="agg")
                      ti = 0
                      for (t0, nt) in win_chunks[w128]:
                        eat = ep.tile([P, G * D], FP, tag="eat")
                        nc.sync.dma_start(
                            out=eat[:, : nt * D], in_=eap_d[:, t0 * D : (t0 + nt) * D]
                        )
                        if i == 0:
                            gx0 = gp.tile([P, G * D], FP, tag="gx0")
                            nc.sync.dma_start(
                                out=gx0[:, : nt * D],
                                in_=xsrc0_d[:, t0 * D : (t0 + nt) * D],
                            )
                            nc.vector.tensor_add(
                                out=eat[:, : nt * D], in0=eat[:, : nt * D],
                                in1=gx0[:, : nt * D],
                            )
                            nc.scalar.activation(
                                out=eat[:, : nt * D], in_=eat[:, : nt * D],
                                func=mybir.ActivationFunctionType.Relu,
                            )
                        else:
                            for t in range(nt):
                                gx = gp.tile([P, D], FP, tag="gx")
                                nc.gpsimd.indirect_dma_start(
                                    out=gx[:], out_offset=None, in_=src_d[:],
                                    in_offset=bass.IndirectOffsetOnAxis(
                                        ap=eidx_t[:, t0 + t : t0 + t + 1], axis=0
                                    ),
                                )
                                nc.vector.tensor_add(
                                    out=eat[:, t * D : (t + 1) * D],
                                    in0=eat[:, t * D : (t + 1) * D], in1=gx[:],
                                )
                                nc.scalar.activation(
                                    out=eat[:, t * D : (t + 1) * D],
                                    in_=eat[:, t * D : (t + 1) * D],
                                    func=mybir.ActivationFunctionType.Relu,
                                )
                        for t in range(nt):
                            S = sp.tile([P, P], FP, tag="S")
                            nc.vector.tensor_tensor(
                                out=S[:],
                                in0=edst_t[:, t0 + t : t0 + t + 1].to_broadcast([P, P]),
                                in1=C["iota128"][:],
                                op=mybir.AluOpType.is_equal,
                            )
                            nc.tensor.matmul(
                                out=pagg[:], lhsT=eat[:, t * D : (t + 1) * D], rhs=S[:],
                                start=(ti == 0), stop=(ti == ntw - 1),
                            )
                            ti += 1
                      if ntw:
                        nc.vector.tensor_add(
                            out=tT[:, sub * P : (sub + 1) * P],
                            in0=xw[:, sub * P : (sub + 1) * P], in1=pagg[:],
                        )
                      else:
                        nc.vector.tensor_copy(
                            out=tT[:, sub * P : (sub + 1) * P],
                            in_=xw[:, sub * P : (sub + 1) * P],
                        )
                    p1 = pmlp_p.tile([P, W], FP, tag="pm")
                    nc.tensor.matmul(
                        out=p1[:], lhsT=C["gcw1"][:, i * D : (i + 1) * D], rhs=tT[:],
                        start=True, stop=True,
                    )
                    h1 = wp.tile([P, W], FP, tag="h1")
                    nc.scalar.activation(
                        out=h1[:], in_=p1[:], func=mybir.ActivationFunctionType.Relu,
                        bias=C["gcb1"][:, i : i + 1],
                    )
                    p2 = pmlp_p.tile([P, W], FP, tag="pm")
                    nc.tensor.matmul(
                        out=p2[:], lhsT=C["gcw2"][:, i * D : (i + 1) * D], rhs=h1[:],
                        start=True, stop=True,
                    )
                    xnT = wp.tile([P, W], FP, tag="xnT")
                    nc.scalar.activation(
                        out=xnT[:], in_=p2[:], func=mybir.ActivationFunctionType.Relu,
                        bias=C["gcb2"][:, i : i + 1],
                    )
                    nc.sync.dma_start(out=xt[i][:, w * W : (w + 1) * W], in_=xnT[:])
                    for q in range(4):
                        pt = ptr_p.tile([P, P], FP, tag="pt")
                        nc.tensor.transpose(
                            out=pt[:], in_=xnT[:, q * P : (q + 1) * P], identity=C["ident"][:]
                        )
                        rows = rp.tile([P, P], FP, tag="rows")
                        nc.vector.tensor_copy(out=rows[:], in_=pt[:])
                        if i < 2:
                            nc.sync.dma_start(
                                out=xr[i][w * W + q * P : w * W + (q + 1) * P, :],
                                in_=rows[:],
                            )
                        col = w * 4 + q
                        Sb = sp.tile([P, B], FP, tag="Sb")
                        nc.vector.tensor_tensor(
                            out=Sb[:],
                            in0=batchv_t[:, col : col + 1].to_broadcast([P, B]),
                            in1=C["iota64"][:],
                            op=mybir.AluOpType.is_equal,
                        )
                        nc.tensor.matmul(
                            out=pxg[:], lhsT=Sb[:], rhs=rows[:],
                            start=(col == 0), stop=(col == NT - 1),
                        )
                nc.vector.tensor_copy(out=xg_sb[:, i * D : (i + 1) * D], in_=pxg[:])
                if i < 2:
                    nc.gpsimd.collective_compute(
                        "AllGather", mybir.AluOpType.bypass, replica_groups=RG,
                        ins=[xr[i][:]], outs=[ag[i][:]],
                    )
            nc.sync.dma_start(out=xg_o[:], in_=xg_sb[:])

            # ---------------- Phase C: motif features ----------------
            pxmh = [pacc_p.tile([P, W], FP, tag="acc" if mw == 0 else "acc2", name=f"pxmh{mw}") for mw in range(2)]
            for w in range(NW):
                pC = pmlp_p.tile([P, W], FP, tag="pm")
                for k in range(NGC):
                    xk = wp.tile([P, W], FP, tag="xw")
                    nc.sync.dma_start(out=xk[:], in_=xt[k][:, w * W : (w + 1) * W])
                    nc.tensor.matmul(
                        out=pC[:], lhsT=C["linw"][:, k * D : (k + 1) * D], rhs=xk[:],
                        start=(k == 0), stop=(k == NGC - 1),
                    )
                xmnT = wp.tile([P, W], FP, tag="h1")
                nc.scalar.activation(
                    out=xmnT[:], in_=pC[:], func=mybir.ActivationFunctionType.Relu,
                    bias=C["linb"][:],
                )
                for q in range(4):
                    pt = ptr_p.tile([P, P], FP, tag="pt")
                    nc.tensor.transpose(
                        out=pt[:], in_=xmnT[:, q * P : (q + 1) * P], identity=C["ident"][:]
                    )
                    rows = rp.tile([P, P], FP, tag="rows")
                    nc.vector.tensor_copy(out=rows[:], in_=pt[:])
                    col = w * 4 + q
                    for mw in range(2):
                        Sm = sp.tile([P, W], FP, tag="S")
                        nc.vector.tensor_tensor(
                            out=Sm[:],
                            in0=n2ml_t[:, col : col + 1].to_broadcast([P, W]),
                            in1=C["iota512" if mw == 0 else "iota512b"][:],
                            op=mybir.AluOpType.is_equal,
                        )
                        nc.tensor.matmul(
                            out=pxmh[mw][:], lhsT=rows[:], rhs=Sm[:],
                            start=(col == 0), stop=(col == NT - 1),
                        )
            xmhT = ap.tile([D, M_pad], FP, tag="xmhT")
            nc.vector.tensor_add(
                out=xmhT[:, :W], in0=pxmh[0][:], in1=membT_t[:, :W]
            )
            if M_pad > W:
                nc.vector.tensor_add(
                    out=xmhT[:, W:M_pad], in0=pxmh[1][:, : M_pad - W],
                    in1=membT_t[:, W:M_pad],
                )
            for mw in range(MW):
                pt = ptr_p.tile([P, P], FP, tag="pt")
                nc.tensor.transpose(
                    out=pt[:], in_=xmhT[:, mw * P : (mw + 1) * P], identity=C["ident"][:]
                )
                rows = rp.tile([P, P], FP, tag="rows")
                nc.vector.tensor_copy(out=rows[:], in_=pt[:])
                nc.sync.dma_start(out=mrows[0][mw * P : (mw + 1) * P, :], in_=rows[:])
            nc.gpsimd.collective_compute(
                "AllGather", mybir.AluOpType.bypass, replica_groups=RG,
                ins=[mrows[0][:]], outs=[agm[0][:]],
            )

            # ---------------- motif GIN layers ----------------
            mwin_chunks = [[] for _ in range(MW)]
            tpos = 0
            for (w, nt) in msched:
                off = 0
                while off < nt:
                    n = min(G, nt - off)
                    mwin_chunks[w].append((tpos + off, n))
                    off += n
                tpos += nt
            mwin_tiles = [sum(c[1] for c in mwin_chunks[w]) for w in range(MW)]

            hT_prev = xmhT
            for i in range(NMC):
                pxm = pacc_p.tile([B, D], FP, tag="acc")
                hT_new = ap.tile([D, M_pad], FP, tag=f"mh{i}T")
                for w in range(MW):
                    ntw = mwin_tiles[w]
                    pagg = pagg_p.tile([P, P], FP, tag="agg")
                    ti = 0
                    for (t0, nt) in mwin_chunks[w]:
                        gm = gp.tile([P, G * D], FP, tag="gx")
                        for t in range(nt):
                            nc.gpsimd.indirect_dma_start(
                                out=gm[:, t * D : (t + 1) * D], out_offset=None,
                                in_=agm[i][:],
                                in_offset=bass.IndirectOffsetOnAxis(
                                    ap=midx_t[:, t0 + t : t0 + t + 1], axis=0
                                ),
                            )
                        for t in range(nt):
                            Sm = sp.tile([P, P], FP, tag="Sm128")
                            nc.vector.tensor_tensor(
                                out=Sm[:],
                                in0=mdst_t[:, t0 + t : t0 + t + 1].to_broadcast([P, P]),
                                in1=C["iota128"][:],
                                op=mybir.AluOpType.is_equal,
                            )
                            nc.tensor.matmul(
                                out=pagg[:], lhsT=gm[:, t * D : (t + 1) * D], rhs=Sm[:],
                                start=(ti == 0), stop=(ti == ntw - 1),
                            )
                            ti += 1
                    tT = wp.tile([P, P], FP, tag="mtT")
                    if ntw:
                        nc.vector.tensor_add(
                            out=tT[:], in0=hT_prev[:, w * P : (w + 1) * P], in1=pagg[:]
                        )
                    else:
                        nc.vector.tensor_copy(out=tT[:], in_=hT_prev[:, w * P : (w + 1) * P])
                    p1 = pmlp_p.tile([P, P], FP, tag="pm")
                    nc.tensor.matmul(
                        out=p1[:], lhsT=C["mcw1"][:, i * D : (i + 1) * D], rhs=tT[:],
                        start=True, stop=True,
                    )
                    h1 = wp.tile([P, P], FP, tag="mh1")
                    nc.scalar.activation(
                        out=h1[:], in_=p1[:], func=mybir.ActivationFunctionType.Relu,
                        bias=C["mcb1"][:, i : i + 1],
                    )
                    p2 = pmlp_p.tile([P, P], FP, tag="pm")
                    nc.tensor.matmul(
                        out=p2[:], lhsT=C["mcw2"][:, i * D : (i + 1) * D], rhs=h1[:],
                        start=True, stop=True,
                    )
                    nc.scalar.activation(
                        out=hT_new[:, w * P : (w + 1) * P], in_=p2[:],
                        func=mybir.ActivationFunctionType.Relu,
                        bias=C["mcb2"][:, i : i + 1],
                    )
                    pt = ptr_p.tile([P, P], FP, tag="pt")
                    nc.tensor.transpose(
                        out=pt[:], in_=hT_new[:, w * P : (w + 1) * P],
                        identity=C["ident"][:],
                    )
                    rows = rp.tile([P, P], FP, tag="rows")
                    nc.vector.tensor_copy(out=rows[:], in_=pt[:])
                    if i == 0:
                        nc.sync.dma_start(
                            out=mrows[1][w * P : (w + 1) * P, :], in_=rows[:]
                        )
                    Smb = sp.tile([P, B], FP, tag="Sb")
                    nc.vector.tensor_tensor(
                        out=Smb[:],
                        in0=mbv_t[:, w : w + 1].to_broadcast([P, B]),
                        in1=C["iota64"][:],
                        op=mybir.AluOpType.is_equal,
                    )
                    nc.tensor.matmul(
                        out=pxm[:], lhsT=Smb[:], rhs=rows[:],
                        start=(w == 0), stop=(w == MW - 1),
                    )
                nc.vector.tensor_copy(out=xm_sb[:, i * D : (i + 1) * D], in_=pxm[:])
                if i == 0:
                    nc.gpsimd.collective_compute(
                        "AllGather", mybir.AluOpType.bypass, replica_groups=RG,
                        ins=[mrows[1][:]], outs=[agm[1][:]],
                    )
                hT_prev = hT_new
            nc.sync.dma_start(out=xm_o[:], in_=xm_sb[:])

    _split_multiwait(nc)
    return nc


def kernel(**inputs):
    consts, per_core, geom = _prep(inputs)
    nc = _build(consts, geom)
    in_maps = []
    for c in range(NCORES):
        m = dict(consts)
        m.update(per_core[c])
        in_maps.append({k: np.ascontiguousarray(v) for k, v in m.items()})
    res = run_bass_kernel_spmd(nc, in_maps, list(range(NCORES)))
    xg = np.zeros((B, NGC * D), np.float32)
    xm = np.zeros((B, NMC * D), np.float32)
    for r in res.results:
        xg += r["xg_part"]
        xm += r["xm_part"]
    kernel._last_exec_ns = res.exec_time_ns
    return xm, xg
